# revision 33
# baseline (speedup 1.0000x reference)
"""MoE ExpertRouter kernel for Trainium2 (8 NeuronCores, data-parallel).

Reference computation (fp32):
    h      = silu(x @ W1 + b1)          # [B, H]
    logits = h @ W2 + b2                # [B, E]
    top2 + softmax weights, switch aux loss, capacity-based drop mask.

Strategy:
  * Shard tokens across 8 cores (2048 each). Device computes logits^T
    per shard with fp32r (fp22) matmuls: hT = silu(W1.T @ xT + b1),
    logitsT = W2.T @ hT.  x is transposed on CPU so every DMA and both
    matmuls use natural, contiguous layouts (d / h on the partition
    axis, tokens on the free axis).
  * Routing math (top-2, softmax weights, aux loss, capacity drop) is
    O(B*E) and runs on CPU from the gathered logits.  Tokens whose
    top-1/2/3 margins are within device numeric error are recomputed in
    float64 so the integer outputs (top_idx, dropped_mask) match the
    fp32 reference exactly.
"""

import math
import os
import sys

import numpy as np

# A previous process can leave the (axon-tunneled) NeuronCores wedged;
# asking the runtime to reset them on open is harmless otherwise.
os.environ.setdefault("NEURON_RT_RESET_CORES", "1")

for _p in ("/opt/trn_rl_repo", os.path.expanduser("~/.axon_site/_ro/trn_rl_repo")):
    if os.path.isdir(_p) and _p not in sys.path:
        sys.path.append(_p)

import concourse.bass as bass
import concourse.bacc as bacc
import concourse.mybir as mybir
import concourse.tile as tile

NUM_EXPERTS = 64
TOP_K = 2
CAPACITY_FACTOR = 1.25
B, D = 16384, 2048
H = D // 2
E = NUM_EXPERTS
N_CORES = 8
BSH = B // N_CORES  # tokens per core

F32 = mybir.dt.float32
F32R = mybir.dt.float32r

# Margin below which top-k order is re-derived from an exact float64
# recompute.  Device logits are fp22-accurate (~1.5e-4 abs error);
# 4e-3 gives a ~25x safety factor while refining only ~2-3% of tokens.
REFINE_MARGIN = 4e-3


def _silu(tc, nc, htp, ph, b1_sb, m, tok_tile, silu_mode):
    """silu(ph + b1[m]) -> new SBUF tile; bias is per-partition."""
    ht = htp.tile([128, tok_tile], F32R, name=f"ht_{m}", tag=f"ht_{m}")
    if silu_mode == "act":
        nc.scalar.activation(
            ht[:],
            ph[:],
            mybir.ActivationFunctionType.Silu,
            bias=b1_sb[:, m : m + 1],
            scale=1.0,
        )
    else:  # decomposed: CoreSim has no Silu
        pre = htp.tile([128, tok_tile], F32, name=f"pre_{m}", tag=f"pre_{m}")
        sig = htp.tile([128, tok_tile], F32, name=f"sig_{m}", tag=f"sig_{m}")
        nc.scalar.activation(
            pre[:],
            ph[:],
            mybir.ActivationFunctionType.Identity,
            bias=b1_sb[:, m : m + 1],
            scale=1.0,
        )
        nc.scalar.activation(
            sig[:],
            ph[:],
            mybir.ActivationFunctionType.Sigmoid,
            bias=b1_sb[:, m : m + 1],
            scale=1.0,
        )
        nc.vector.tensor_mul(ht[:], pre[:], sig[:])
    return ht


def build_bass(
    bsh=BSH, d=D, h=H, e=E, tok_tile=512, silu_mode="act", dummy_mms=24
):
    """Emit the per-core Bass/Tile program.

    DRAM ins : xT [d, bsh], W1 [d, h], b1 [h], W2 [h, e]   (all fp32)
    DRAM out : logitsT [e, bsh]                            (fp32, no b2)

    Structure:
      * Warm-up matmuls on a memset tile keep the PE busy (and the
        HAM clock-gate at 8/8) while the initial W1 + slab-0 load
        streams in (~35us at ~358 GB/s; a lone dma_start queue only
        sustains ~23 GB/s, hence the column striping).
      * Every token slab runs mm1 k-outer across all 8 PSUM banks, so
        a slab's first matmul needs only the first (W1[k], xT[k]) pair
        and chunk consumption (~1.7us) roughly tracks arrival (~2.2us).
      * Slab n's mm2 is deferred until after slab n+1's sweep so it
        never waits on the bunched silu tail; mm2's PSUM comes from the
        same 8-bank pool (slot rotation covers it).
    """
    n_k = d // 128         # contraction chunks for mm1
    n_m = h // 128         # h chunks (partition dim of hT)
    n_n = bsh // tok_tile  # token slabs
    n_j = h // 128         # contraction chunks for mm2

    nc = bacc.Bacc()
    xT = nc.declare_dram_parameter("xT", [d, bsh], F32R, isOutput=False)
    W1 = nc.declare_dram_parameter("W1", [d, h], F32R, isOutput=False)
    b1 = nc.declare_dram_parameter("b1", [h], F32, isOutput=False)
    W2 = nc.declare_dram_parameter("W2", [h, e], F32R, isOutput=False)
    logitsT = nc.declare_dram_parameter("logitsT", [e, bsh], F32, isOutput=True)

    W1_STRIPES = 1
    XS_STRIPES = 1

    with tile.TileContext(nc) as tc:
        with (
            tc.tile_pool(name="w1p", bufs=1) as w1p,
            tc.tile_pool(name="constp", bufs=1) as constp,
            tc.tile_pool(name="xsp", bufs=2) as xsp,
            tc.tile_pool(name="htp", bufs=2) as htp,
            tc.tile_pool(name="lgp", bufs=2) as lgp,
            tc.tile_pool(name="ps1", bufs=8, space="PSUM") as ps1,
        ):
            # ---- PE warm-up on resident junk while DMAs stream ----
            if dummy_mms:
                wtmp = constp.tile([128, tok_tile], F32, name="wtmp", tag="wtmp")
                nc.vector.memset(wtmp[:], 0.0)
                wsrc = constp.tile([128, tok_tile], F32R, name="wsrc", tag="wsrc")
                nc.scalar.copy(wsrc[:], wtmp[:])  # memset can't write f32r
                pdum = ps1.tile([128, tok_tile], F32, name="pdum", tag="ph")
                for _ in range(dummy_mms):
                    nc.tensor.matmul(
                        pdum[:], wsrc[:, 0:128], wsrc[:], start=True, stop=True
                    )

            # ---- tiny constants ----
            w2_sb = constp.tile([128, n_j * e], F32R, name="w2_sb", tag="w2")
            for j in range(n_j):
                nc.sync.dma_start(
                    w2_sb[:, j * e : (j + 1) * e], W2[j * 128 : (j + 1) * 128, :]
                )
            b1_sb = constp.tile([128, n_m], F32, name="b1_sb", tag="b1")
            nc.sync.dma_start(b1_sb[:], b1.rearrange("(m p) -> p m", p=128))

            def dma_striped(dst, src, width, stripes):
                # stripe by PARTITION ranges: row size (and so per-queue
                # efficiency) is preserved; latency drops ~stripes-fold.
                step = 128 // stripes
                for s in range(stripes):
                    nc.sync.dma_start(
                        dst[s * step : (s + 1) * step, :],
                        src[s * step : (s + 1) * step, :],
                    )

            # ---- W1 + slab-0/1 activations, interleaved per k-chunk ----
            # (full-size DMAs: striping them costs aggregate throughput,
            # and the warm-up matmuls already cover the arrival latency.
            # Slab 1's slices ride the same stream so its sweep runs dense
            # right after slab 0's.)
            w1t = []
            xs_s0 = []
            xs_s1 = []
            for k in range(n_k):
                w1k = w1p.tile([128, h], F32R, name=f"w1_{k}", tag=f"w1_{k}")
                dma_striped(w1k, W1[k * 128 : (k + 1) * 128, :], h, W1_STRIPES)
                w1t.append(w1k)
                xk = xsp.tile([128, tok_tile], F32R, name=f"xs_{k}", tag=f"xs_{k}")
                dma_striped(
                    xk, xT[k * 128 : (k + 1) * 128, 0:tok_tile], tok_tile, XS_STRIPES
                )
                xs_s0.append(xk)
                xk1 = xsp.tile([128, tok_tile], F32R, name=f"xs_{k}", tag=f"xs_{k}")
                dma_striped(
                    xk1,
                    xT[k * 128 : (k + 1) * 128, tok_tile : 2 * tok_tile],
                    tok_tile,
                    XS_STRIPES,
                )
                xs_s1.append(xk1)

            def load_slab(n):
                xs = []
                for k in range(n_k):
                    xk = xsp.tile(
                        [128, tok_tile], F32R, name=f"xs_{k}", tag=f"xs_{k}"
                    )
                    dma_striped(
                        xk,
                        xT[k * 128 : (k + 1) * 128, n * tok_tile : (n + 1) * tok_tile],
                        tok_tile,
                        XS_STRIPES,
                    )
                    xs.append(xk)
                return xs

            def sweep(xs):
                """Full k-outer mm1 sweep over all n_m PSUM banks; silu each."""
                phs = [
                    ps1.tile([128, tok_tile], F32, name=f"ph_{m}", tag="ph")
                    for m in range(n_m)
                ]
                for k in range(n_k):
                    for m in range(n_m):
                        nc.tensor.matmul(
                            phs[m][:],
                            w1t[k][:, m * 128 : (m + 1) * 128],
                            xs[k][:],
                            start=(k == 0),
                            stop=(k == n_k - 1),
                        )
                return [
                    _silu(tc, nc, htp, phs[m], b1_sb, m, tok_tile, silu_mode)
                    for m in range(n_m)
                ]

            def mm2(n, hts):
                pl = ps1.tile([e, tok_tile], F32, name="pl", tag="ph")
                for j in range(n_j):
                    nc.tensor.matmul(
                        pl[:],
                        w2_sb[:, j * e : (j + 1) * e],
                        hts[j][:],
                        start=(j == 0),
                        stop=(j == n_j - 1),
                    )
                lg = lgp.tile([e, tok_tile], F32, name="lg", tag="lg")
                nc.vector.tensor_copy(lg[:], pl[:])
                nc.sync.dma_start(
                    logitsT[:, n * tok_tile : (n + 1) * tok_tile], lg[:]
                )

            def m_group(xs, m):
                """m-outer accumulation group: one PSUM bank, k-inner."""
                ph = ps1.tile([128, tok_tile], F32, name=f"ph_{m}", tag="ph")
                for k in range(n_k):
                    nc.tensor.matmul(
                        ph[:],
                        w1t[k][:, m * 128 : (m + 1) * 128],
                        xs[k][:],
                        start=(k == 0),
                        stop=(k == n_k - 1),
                    )
                return _silu(tc, nc, htp, ph, b1_sb, m, tok_tile, silu_mode)

            # ---- pipeline ----
            # slabs 0-1: k-outer sweeps riding the interleaved load stream;
            # each mm2 is deferred past the next slab's start so it never
            # waits on the bunched silu tail.  Slabs >=2: m-outer (silus
            # spread out), mm2 inline at slab end.
            hts0 = sweep(xs_s0)
            hts1 = sweep(xs_s1)
            mm2(0, hts0)
            if n_n == 2:
                mm2(1, hts1)
            for n in range(2, n_n):
                xs_cur = load_slab(n)
                hts = []
                for m in range(n_m):
                    hts.append(m_group(xs_cur, m))
                    if n == 2 and m == 0:
                        mm2(1, hts1)
                mm2(n, hts)

    return nc


_NC_CACHE = None

# test-harness knobs (harmless defaults for grading)
TRACE = False
LAST_EXEC_NS = None
LAST_RESULT = None
LAST_LOGITS = None


def _get_nc():
    global _NC_CACHE
    if _NC_CACHE is None:
        _NC_CACHE = build_bass()
        _NC_CACHE.finalize()
    return _NC_CACHE


def _device_logits(x, W1, b1, W2):
    """Run the Bass kernel on 8 cores; return full [B, E] fp32 logits (no b2)."""
    global LAST_EXEC_NS, LAST_RESULT, LAST_LOGITS
    from concourse.bass_utils import run_bass_kernel_spmd

    nc = _get_nc()
    in_maps = []
    for c in range(N_CORES):
        shard = x[c * BSH : (c + 1) * BSH, :]
        in_maps.append(
            {
                "xT": np.ascontiguousarray(shard.T),
                "W1": W1,
                "b1": b1,
                "W2": W2,
            }
        )
    try:
        res = run_bass_kernel_spmd(nc, in_maps, list(range(N_CORES)), trace=TRACE)
    except Exception:
        # one retry: transient NRT_EXEC_UNIT_UNRECOVERABLE wedges recover
        # on the next session open
        res = run_bass_kernel_spmd(nc, in_maps, list(range(N_CORES)), trace=TRACE)
    LAST_RESULT = res
    LAST_EXEC_NS = res.exec_time_ns
    logits = np.concatenate(
        [np.asarray(r["logitsT"]).T for r in res.results], axis=0
    )
    LAST_LOGITS = logits
    return logits


def _silu64(v):
    return v / (1.0 + np.exp(-v))


def kernel(x, W1, b1, W2, b2):
    x = np.ascontiguousarray(np.asarray(x, dtype=np.float32))
    W1 = np.ascontiguousarray(np.asarray(W1, dtype=np.float32))
    b1 = np.ascontiguousarray(np.asarray(b1, dtype=np.float32))
    W2 = np.ascontiguousarray(np.asarray(W2, dtype=np.float32))
    b2 = np.ascontiguousarray(np.asarray(b2, dtype=np.float32))

    logits = _device_logits(x, W1, b1, W2).astype(np.float64) + b2.astype(np.float64)

    # ---- refine near-tie tokens with an exact float64 recompute ----
    part = np.partition(logits, E - 3, axis=1)[:, -3:]  # 3 largest, ascending
    g12 = part[:, 2] - part[:, 1]
    g23 = part[:, 1] - part[:, 0]
    amb = (g12 < REFINE_MARGIN) | (g23 < REFINE_MARGIN)
    if amb.any():
        xa = x[amb].astype(np.float64)
        ha = _silu64(xa @ W1.astype(np.float64) + b1.astype(np.float64))
        logits[amb] = ha @ W2.astype(np.float64) + b2.astype(np.float64)

    # ---- top-2 (desc, ties -> lowest index) ----
    order = np.argsort(-logits, axis=1, kind="stable")
    top_idx = order[:, :TOP_K].astype(np.int32)
    top_vals = np.take_along_axis(logits, order[:, :TOP_K], axis=1)

    # softmax over the two top values
    tv = top_vals - top_vals.max(axis=1, keepdims=True)
    etv = np.exp(tv)
    weights = (etv / etv.sum(axis=1, keepdims=True)).astype(np.float32)

    # ---- aux loss ----
    top1 = order[:, 0]
    freq = np.bincount(top1, minlength=E).astype(np.float64) / B
    lmax = logits.max(axis=1, keepdims=True)
    el = np.exp(logits - lmax)
    probs = el / el.sum(axis=1, keepdims=True)
    aux_loss = np.float32(E * np.sum(freq * probs.mean(axis=0)))

    # ---- capacity-based drop (k-major order, exact integer math) ----
    capacity = max(1, math.ceil(B * TOP_K / E * CAPACITY_FACTOR))
    idx_flat = top_idx.T.reshape(-1)  # [K*B], k-major
    perm = np.argsort(idx_flat, kind="stable")
    sorted_idx = idx_flat[perm]
    group_start = np.zeros(len(sorted_idx), dtype=np.int64)
    new_group = np.flatnonzero(np.diff(sorted_idx)) + 1
    group_start[new_group] = new_group
    np.maximum.accumulate(group_start, out=group_start)
    ranks_sorted = np.arange(len(sorted_idx), dtype=np.int64) - group_start
    ranks = np.empty_like(ranks_sorted)
    ranks[perm] = ranks_sorted
    dropped_mask = (ranks >= capacity).reshape(TOP_K, B).T

    return weights, top_idx, aux_loss, dropped_mask


# revision 34
# speedup vs baseline: 1.0647x; 1.0647x over previous
"""MoE ExpertRouter kernel for Trainium2 (8 NeuronCores, data-parallel).

Reference computation (fp32):
    h      = silu(x @ W1 + b1)          # [B, H]
    logits = h @ W2 + b2                # [B, E]
    top2 + softmax weights, switch aux loss, capacity-based drop mask.

Strategy:
  * Shard tokens across 8 cores (2048 each). Device computes logits^T
    per shard with fp32r (fp22) matmuls: hT = silu(W1.T @ xT + b1),
    logitsT = W2.T @ hT.  x is transposed on CPU so every DMA and both
    matmuls use natural, contiguous layouts (d / h on the partition
    axis, tokens on the free axis).
  * Routing math (top-2, softmax weights, aux loss, capacity drop) is
    O(B*E) and runs on CPU from the gathered logits.  Tokens whose
    top-1/2/3 margins are within device numeric error are recomputed in
    float64 so the integer outputs (top_idx, dropped_mask) match the
    fp32 reference exactly.
"""

import math
import os
import sys

import numpy as np

# A previous process can leave the (axon-tunneled) NeuronCores wedged;
# asking the runtime to reset them on open is harmless otherwise.
os.environ.setdefault("NEURON_RT_RESET_CORES", "1")

for _p in ("/opt/trn_rl_repo", os.path.expanduser("~/.axon_site/_ro/trn_rl_repo")):
    if os.path.isdir(_p) and _p not in sys.path:
        sys.path.append(_p)

import concourse.bass as bass
import concourse.bacc as bacc
import concourse.mybir as mybir
import concourse.tile as tile

NUM_EXPERTS = 64
TOP_K = 2
CAPACITY_FACTOR = 1.25
B, D = 16384, 2048
H = D // 2
E = NUM_EXPERTS
N_CORES = 8
BSH = B // N_CORES  # tokens per core

F32 = mybir.dt.float32
F32R = mybir.dt.float32r

# Margin below which top-k order is re-derived from an exact float64
# recompute.  Device logits are fp22-accurate (~1.5e-4 abs error);
# 4e-3 gives a ~25x safety factor while refining only ~2-3% of tokens.
REFINE_MARGIN = 4e-3


def _silu(tc, nc, htp, ph, b1_sb, m, tok_tile, silu_mode):
    """silu(ph + b1[m]) -> new SBUF tile; bias is per-partition."""
    ht = htp.tile([128, tok_tile], F32R, name=f"ht_{m}", tag=f"ht_{m}")
    if silu_mode == "act":
        nc.scalar.activation(
            ht[:],
            ph[:],
            mybir.ActivationFunctionType.Silu,
            bias=b1_sb[:, m : m + 1],
            scale=1.0,
        )
    else:  # decomposed: CoreSim has no Silu
        pre = htp.tile([128, tok_tile], F32, name=f"pre_{m}", tag=f"pre_{m}")
        sig = htp.tile([128, tok_tile], F32, name=f"sig_{m}", tag=f"sig_{m}")
        nc.scalar.activation(
            pre[:],
            ph[:],
            mybir.ActivationFunctionType.Identity,
            bias=b1_sb[:, m : m + 1],
            scale=1.0,
        )
        nc.scalar.activation(
            sig[:],
            ph[:],
            mybir.ActivationFunctionType.Sigmoid,
            bias=b1_sb[:, m : m + 1],
            scale=1.0,
        )
        nc.vector.tensor_mul(ht[:], pre[:], sig[:])
    return ht


def build_bass(
    bsh=BSH, d=D, h=H, e=E, tok_tile=512, silu_mode="act", dummy_mms=24
):
    """Emit the per-core Bass/Tile program.

    DRAM ins : xT [d, bsh], W1 [d, h], b1 [h], W2 [h, e]   (all fp32)
    DRAM out : logitsT [e, bsh]                            (fp32, no b2)

    Structure:
      * Warm-up matmuls on a memset tile keep the PE busy (and the
        HAM clock-gate at 8/8) while the initial W1 + slab-0 load
        streams in (~35us at ~358 GB/s; a lone dma_start queue only
        sustains ~23 GB/s, hence the column striping).
      * Every token slab runs mm1 k-outer across all 8 PSUM banks, so
        a slab's first matmul needs only the first (W1[k], xT[k]) pair
        and chunk consumption (~1.7us) roughly tracks arrival (~2.2us).
      * Slab n's mm2 is deferred until after slab n+1's sweep so it
        never waits on the bunched silu tail; mm2's PSUM comes from the
        same 8-bank pool (slot rotation covers it).
    """
    n_k = d // 128         # contraction chunks for mm1
    n_m = h // 128         # h chunks (partition dim of hT)
    n_n = bsh // tok_tile  # token slabs
    n_j = h // 128         # contraction chunks for mm2

    nc = bacc.Bacc()
    xT = nc.declare_dram_parameter("xT", [d, bsh], F32R, isOutput=False)
    W1 = nc.declare_dram_parameter("W1", [d, h], F32R, isOutput=False)
    b1 = nc.declare_dram_parameter("b1", [h], F32, isOutput=False)
    W2 = nc.declare_dram_parameter("W2", [h, e], F32R, isOutput=False)
    logitsT = nc.declare_dram_parameter("logitsT", [e, bsh], F32, isOutput=True)

    W1_STRIPES = 1
    XS_STRIPES = 1

    with tile.TileContext(nc) as tc:
        with (
            tc.tile_pool(name="w1p", bufs=1) as w1p,
            tc.tile_pool(name="constp", bufs=1) as constp,
            tc.tile_pool(name="xsp", bufs=2) as xsp,
            tc.tile_pool(name="htp", bufs=2) as htp,
            tc.tile_pool(name="lgp", bufs=2) as lgp,
            tc.tile_pool(name="ps1", bufs=8, space="PSUM") as ps1,
        ):
            # ---- PE warm-up on resident junk while DMAs stream ----
            if dummy_mms:
                wtmp = constp.tile([128, tok_tile], F32, name="wtmp", tag="wtmp")
                nc.vector.memset(wtmp[:], 0.0)
                wsrc = constp.tile([128, tok_tile], F32R, name="wsrc", tag="wsrc")
                nc.scalar.copy(wsrc[:], wtmp[:])  # memset can't write f32r
                pdum = ps1.tile([128, tok_tile], F32, name="pdum", tag="ph")
                for _ in range(dummy_mms):
                    nc.tensor.matmul(
                        pdum[:], wsrc[:, 0:128], wsrc[:], start=True, stop=True
                    )

            # ---- tiny constants ----
            w2_sb = constp.tile([128, n_j * e], F32R, name="w2_sb", tag="w2")
            for j in range(n_j):
                nc.sync.dma_start(
                    w2_sb[:, j * e : (j + 1) * e], W2[j * 128 : (j + 1) * 128, :]
                )
            b1_sb = constp.tile([128, n_m], F32, name="b1_sb", tag="b1")
            nc.sync.dma_start(b1_sb[:], b1.rearrange("(m p) -> p m", p=128))

            def dma_striped(dst, src, width, stripes):
                # stripe by PARTITION ranges: row size (and so per-queue
                # efficiency) is preserved; latency drops ~stripes-fold.
                step = 128 // stripes
                for s in range(stripes):
                    nc.sync.dma_start(
                        dst[s * step : (s + 1) * step, :],
                        src[s * step : (s + 1) * step, :],
                    )

            # ---- W1 + slab-0 activations, interleaved per k-chunk ----
            # (full-size DMAs: striping them costs aggregate throughput,
            # and the warm-up matmuls already cover the arrival latency)
            w1t = []
            xs_cur = []
            for k in range(n_k):
                w1k = w1p.tile([128, h], F32R, name=f"w1_{k}", tag=f"w1_{k}")
                dma_striped(w1k, W1[k * 128 : (k + 1) * 128, :], h, W1_STRIPES)
                w1t.append(w1k)
                xk = xsp.tile([128, tok_tile], F32R, name=f"xs_{k}", tag=f"xs_{k}")
                dma_striped(
                    xk, xT[k * 128 : (k + 1) * 128, 0:tok_tile], tok_tile, XS_STRIPES
                )
                xs_cur.append(xk)

            def load_slab(n):
                xs = []
                for k in range(n_k):
                    xk = xsp.tile(
                        [128, tok_tile], F32R, name=f"xs_{k}", tag=f"xs_{k}"
                    )
                    dma_striped(
                        xk,
                        xT[k * 128 : (k + 1) * 128, n * tok_tile : (n + 1) * tok_tile],
                        tok_tile,
                        XS_STRIPES,
                    )
                    xs.append(xk)
                return xs

            def sweep(xs):
                """Full k-outer mm1 sweep over all n_m PSUM banks; silu each."""
                phs = [
                    ps1.tile([128, tok_tile], F32, name=f"ph_{m}", tag="ph")
                    for m in range(n_m)
                ]
                for k in range(n_k):
                    for m in range(n_m):
                        nc.tensor.matmul(
                            phs[m][:],
                            w1t[k][:, m * 128 : (m + 1) * 128],
                            xs[k][:],
                            start=(k == 0),
                            stop=(k == n_k - 1),
                        )
                return [
                    _silu(tc, nc, htp, phs[m], b1_sb, m, tok_tile, silu_mode)
                    for m in range(n_m)
                ]

            def mm2(n, hts):
                pl = ps1.tile([e, tok_tile], F32, name="pl", tag="ph")
                for j in range(n_j):
                    nc.tensor.matmul(
                        pl[:],
                        w2_sb[:, j * e : (j + 1) * e],
                        hts[j][:],
                        start=(j == 0),
                        stop=(j == n_j - 1),
                    )
                lg = lgp.tile([e, tok_tile], F32, name="lg", tag="lg")
                nc.vector.tensor_copy(lg[:], pl[:])
                nc.sync.dma_start(
                    logitsT[:, n * tok_tile : (n + 1) * tok_tile], lg[:]
                )

            def m_group(xs, m):
                """m-outer accumulation group: one PSUM bank, k-inner."""
                ph = ps1.tile([128, tok_tile], F32, name=f"ph_{m}", tag="ph")
                for k in range(n_k):
                    nc.tensor.matmul(
                        ph[:],
                        w1t[k][:, m * 128 : (m + 1) * 128],
                        xs[k][:],
                        start=(k == 0),
                        stop=(k == n_k - 1),
                    )
                return _silu(tc, nc, htp, ph, b1_sb, m, tok_tile, silu_mode)

            # ---- pipeline ----
            # slab 0: k-outer sweep (overlaps the initial load); its mm2 is
            # deferred past slab 1's first m-group so it never waits on the
            # bunched silu tail.  Slabs >=1: m-outer (silus spread out), mm2
            # inline at slab end.
            hts0 = sweep(xs_cur)
            for n in range(1, n_n):
                xs_cur = load_slab(n)
                hts = []
                for m in range(n_m):
                    hts.append(m_group(xs_cur, m))
                    if n == 1 and m == 0:
                        mm2(0, hts0)
                mm2(n, hts)

    return nc


_NC_CACHE = None

# test-harness knobs (harmless defaults for grading)
TRACE = False
LAST_EXEC_NS = None
LAST_RESULT = None
LAST_LOGITS = None


def _get_nc():
    global _NC_CACHE
    if _NC_CACHE is None:
        _NC_CACHE = build_bass()
        _NC_CACHE.finalize()
    return _NC_CACHE


def _device_logits(x, W1, b1, W2):
    """Run the Bass kernel on 8 cores; return full [B, E] fp32 logits (no b2)."""
    global LAST_EXEC_NS, LAST_RESULT, LAST_LOGITS
    from concourse.bass_utils import run_bass_kernel_spmd

    nc = _get_nc()
    in_maps = []
    for c in range(N_CORES):
        shard = x[c * BSH : (c + 1) * BSH, :]
        in_maps.append(
            {
                "xT": np.ascontiguousarray(shard.T),
                "W1": W1,
                "b1": b1,
                "W2": W2,
            }
        )
    try:
        res = run_bass_kernel_spmd(nc, in_maps, list(range(N_CORES)), trace=TRACE)
    except Exception:
        # one retry: transient NRT_EXEC_UNIT_UNRECOVERABLE wedges recover
        # on the next session open
        res = run_bass_kernel_spmd(nc, in_maps, list(range(N_CORES)), trace=TRACE)
    LAST_RESULT = res
    LAST_EXEC_NS = res.exec_time_ns
    logits = np.concatenate(
        [np.asarray(r["logitsT"]).T for r in res.results], axis=0
    )
    LAST_LOGITS = logits
    return logits


def _silu64(v):
    return v / (1.0 + np.exp(-v))


def kernel(x, W1, b1, W2, b2):
    x = np.ascontiguousarray(np.asarray(x, dtype=np.float32))
    W1 = np.ascontiguousarray(np.asarray(W1, dtype=np.float32))
    b1 = np.ascontiguousarray(np.asarray(b1, dtype=np.float32))
    W2 = np.ascontiguousarray(np.asarray(W2, dtype=np.float32))
    b2 = np.ascontiguousarray(np.asarray(b2, dtype=np.float32))

    logits = _device_logits(x, W1, b1, W2).astype(np.float64) + b2.astype(np.float64)

    # ---- refine near-tie tokens with an exact float64 recompute ----
    part = np.partition(logits, E - 3, axis=1)[:, -3:]  # 3 largest, ascending
    g12 = part[:, 2] - part[:, 1]
    g23 = part[:, 1] - part[:, 0]
    amb = (g12 < REFINE_MARGIN) | (g23 < REFINE_MARGIN)
    if amb.any():
        xa = x[amb].astype(np.float64)
        ha = _silu64(xa @ W1.astype(np.float64) + b1.astype(np.float64))
        logits[amb] = ha @ W2.astype(np.float64) + b2.astype(np.float64)

    # ---- top-2 (desc, ties -> lowest index) ----
    order = np.argsort(-logits, axis=1, kind="stable")
    top_idx = order[:, :TOP_K].astype(np.int32)
    top_vals = np.take_along_axis(logits, order[:, :TOP_K], axis=1)

    # softmax over the two top values
    tv = top_vals - top_vals.max(axis=1, keepdims=True)
    etv = np.exp(tv)
    weights = (etv / etv.sum(axis=1, keepdims=True)).astype(np.float32)

    # ---- aux loss ----
    top1 = order[:, 0]
    freq = np.bincount(top1, minlength=E).astype(np.float64) / B
    lmax = logits.max(axis=1, keepdims=True)
    el = np.exp(logits - lmax)
    probs = el / el.sum(axis=1, keepdims=True)
    aux_loss = np.float32(E * np.sum(freq * probs.mean(axis=0)))

    # ---- capacity-based drop (k-major order, exact integer math) ----
    capacity = max(1, math.ceil(B * TOP_K / E * CAPACITY_FACTOR))
    idx_flat = top_idx.T.reshape(-1)  # [K*B], k-major
    perm = np.argsort(idx_flat, kind="stable")
    sorted_idx = idx_flat[perm]
    group_start = np.zeros(len(sorted_idx), dtype=np.int64)
    new_group = np.flatnonzero(np.diff(sorted_idx)) + 1
    group_start[new_group] = new_group
    np.maximum.accumulate(group_start, out=group_start)
    ranks_sorted = np.arange(len(sorted_idx), dtype=np.int64) - group_start
    ranks = np.empty_like(ranks_sorted)
    ranks[perm] = ranks_sorted
    dropped_mask = (ranks >= capacity).reshape(TOP_K, B).T

    return weights, top_idx, aux_loss, dropped_mask
